# revision 1
# baseline (speedup 1.0000x reference)
"""DGCNN (3x DynamicEdgeConv + global max pool + FC) Trainium2 Bass kernel.

Sharding: data-parallel over graphs. 32 graphs / 8 NeuronCores = 4 graphs/core.
Weights replicated. Each core returns its [128, 4] (feature-major) FC output.

Per-graph algorithm (feature-major [C, P] layout end to end):
  - kNN ranking matrix F = X^T X - 0.5*|x_j|^2 via one PE matmul with the
    lhs=[X;ones], rhs=[X;-0.5 sq] augmentation (top-20 largest F == 20-NN).
  - Top-20 indices per node: 3 rounds of DVE max8 / max_index / match_replace.
  - EdgeConv decomposes: relu(max_k([x_i, x_j-x_i] W + b))
      = relu((Wtop-Wbot)^T x_i + max_k Wbot^T x_j + b)  (relu/max commute).
    So per node: A = Wd^T X (PE), Bm = X^T Wbot rows in DRAM, M = max over the
    20 neighbor rows via 20 indirect DMA gathers with max-accumulate.
  - h^T = relu(transpose(M) + A + b) using PE transpose + matmul accumulated
    into one PSUM tile, ACT applies relu+bias.

HW note: multi-column indirect-DMA offset APs are consumed in a scrambled
order on this hardware, so each gather uses a [128, 1] offset column (one
descriptor per partition — unambiguous, production-tested shape): 20 gathers
per 128-node row-tile into k-slices of a [128, 20, H] tile, then one DVE
tensor_reduce(max) over k.
"""
import sys

sys.path.insert(0, "/opt/trn_rl_repo")
import numpy as np
import concourse.bass as bass
import concourse.bacc as bacc
import concourse.mybir as mybir
from concourse.bass_utils import run_bass_kernel_spmd
from concourse.tile import TileContext
from concourse import masks

B, P, KNN = 32, 1024, 20
NCORES, GPC = 8, 4
NEG = -3.0e38
f32, u32 = mybir.dt.float32, mybir.dt.uint32
RELU = mybir.ActivationFunctionType.Relu
COPY = mybir.ActivationFunctionType.Copy
MAX = mybir.AluOpType.max
DIMS = {1: (3, 64), 2: (64, 64), 3: (64, 128)}

_cache = {}


def _emit_layer(nc, tc, pools, W, state, g, l, is_last):
    C, H = DIMS[l]
    lhs, rhs = state[(g, "lhs")], state[(g, "rhs")]
    wd, wb, bl = W[f"wd{l}"], W[f"wb{l}"], W[f"b{l}"]
    ident, diagneg, onescol = W["ident"], W["diagneg"], W["onescol"]
    psF, psT, psB = pools["psF"], pools["psT"], pools["psB"]
    pool = pools["sbuf"]
    bm_dram = state[(g, "bm64")] if H == 64 else state[(g, "bm128")]

    # ---- 1. ones row + sq row (layer 1 rows are shipped from host) ----
    if l > 1:
        _emit_sq_prep(nc, pools, W, lhs, rhs, C)
    _emit_layer_rest(nc, tc, pools, W, state, g, l, is_last)


def _emit_sq_prep(nc, pools, W, lhs, rhs, C):
    psF = pools["psF"]
    pool = pools["sbuf"]
    onescol = W["onescol"]
    nc.vector.memset(lhs[C:C + 1, :], 1.0)
    x2 = pool.tile([C, P], f32, tag="x2", bufs=1)
    nc.scalar.square(x2[0:C, :], lhs[0:C, :])
    for jb in range(2):
        psq = psF.tile([128, 512], f32, tag="psF")
        nc.tensor.matmul(psq[0:1, :], onescol[0:C, :],
                         x2[0:C, 512 * jb:512 * (jb + 1)], start=True, stop=True)
        nc.scalar.activation(rhs[C:C + 1, 512 * jb:512 * (jb + 1)], psq[0:1, :],
                             COPY, scale=-0.5)


def _emit_layer_rest(nc, tc, pools, W, state, g, l, is_last):
    C, H = DIMS[l]
    lhs, rhs = state[(g, "lhs")], state[(g, "rhs")]
    wd, wb, bl = W[f"wd{l}"], W[f"wb{l}"], W[f"b{l}"]
    ident, diagneg, onescol = W["ident"], W["diagneg"], W["onescol"]
    psF, psT, psB = pools["psF"], pools["psT"], pools["psB"]
    pool = pools["sbuf"]
    bm_dram = state[(g, "bm64")] if H == 64 else state[(g, "bm128")]

    # ---- 2. Bm = X^T Wbot, node-major to DRAM ----
    bmt = pool.tile([128, 8, 128], f32, tag="bm", bufs=2)
    for t in range(8):
        pb = psB.tile([128, 128], f32, tag="psB")
        nc.tensor.matmul(pb[:, 0:H], lhs[0:C, 128 * t:128 * (t + 1)], wb[0:C, 0:H],
                         start=True, stop=True)
        nc.scalar.activation(bmt[:, t, 0:H], pb[:, 0:H], COPY)
    nc.sync.dma_start(out=bm_dram[:].rearrange("(t p) h -> p t h", p=128), in_=bmt[:, :, 0:H])

    # ---- 3. F + top-20 indices per node-tile ----
    idxs = pool.tile([128, 8, 24], u32, tag="idx", bufs=3)
    for t in range(8):
        Fsb = pool.tile([128, P], f32, tag="F", bufs=6)
        for jb in range(2):
            fps = psF.tile([128, 512], f32, tag="psF")
            nc.tensor.matmul(fps[:], lhs[0:C + 1, 128 * t:128 * (t + 1)],
                             rhs[0:C + 1, 512 * jb:512 * (jb + 1)],
                             start=True, stop=True)
            nc.scalar.activation(Fsb[:, 512 * jb:512 * (jb + 1)], fps[:], COPY)
        nc.vector.tensor_add(Fsb[:, 128 * t:128 * (t + 1)],
                             Fsb[:, 128 * t:128 * (t + 1)], diagneg[:])
        for r in range(3):
            m8 = pool.tile([128, 8], f32, tag="m8", bufs=4)
            nc.vector.max(out=m8, in_=Fsb)
            nc.vector.max_index(out=idxs[:, t, 8 * r:8 * r + 8], in_max=m8,
                                in_values=Fsb)
            if r < 2:
                nc.vector.match_replace(out=Fsb, in_to_replace=m8, in_values=Fsb,
                                        imm_value=NEG)

    # ---- 4+5. per-row-tile: 20 single-descriptor-per-partition gathers ----
    if is_last:
        h3 = pool.tile([128, P], f32, tag="h3", bufs=1)
        dst = h3
    else:
        Cn = H + 1
        lhs_n = pool.tile([Cn, P], f32, tag=f"lhs{l + 1}", bufs=4)
        rhs_n = pool.tile([Cn, P], f32, tag=f"rhs{l + 1}", bufs=4)
        dst = lhs_n
    for t in range(8):
        wt = pool.tile([128, 24], u32, tag="wt", bufs=6, name=f"wt{t}")
        nc.vector.tensor_copy(wt[:], idxs[:, t, :])
        gt = pool.tile([128, KNN, H], f32, tag="gt", bufs=4, name=f"gt{t}")
        for k in range(KNN):
            nc.gpsimd.indirect_dma_start(
                out=gt[:, k, :], out_offset=None,
                in_=bm_dram[:, :],
                in_offset=bass.IndirectOffsetOnAxis(ap=wt[:, k:k + 1], axis=0),
                bounds_check=P - 1, oob_is_err=False)
        Mt = pool.tile([128, H], f32, tag="Mt", bufs=6, name=f"Mt{t}")
        nc.vector.tensor_reduce(
            out=Mt[:], in_=gt[:].rearrange("p c h -> p h c"),
            axis=mybir.AxisListType.X, op=MAX)
        pt = psT.tile([128, 128], f32, tag="psT")
        nc.tensor.matmul(pt[0:H, :], Mt[:], ident[:], is_transpose=True,
                         start=True, stop=False)
        nc.tensor.matmul(pt[0:H, :], wd[0:C, 0:H],
                         lhs[0:C, 128 * t:128 * (t + 1)], start=False, stop=True)
        nc.scalar.activation(dst[0:H, 128 * t:128 * (t + 1)], pt[0:H, :], RELU,
                             bias=bl[0:H, :])

    if is_last:
        nc.vector.tensor_reduce(out=W["pooledT"][:, g:g + 1], in_=h3[:],
                                axis=mybir.AxisListType.X, op=MAX)
    else:
        nc.scalar.activation(rhs_n[0:H, 0:512], lhs_n[0:H, 0:512], COPY)
        nc.scalar.activation(rhs_n[0:H, 512:P], lhs_n[0:H, 512:P], COPY)
        state[(g, "lhs")], state[(g, "rhs")] = lhs_n, rhs_n


def _build():
    nc = bacc.Bacc("TRN2", target_bir_lowering=False, debug=False,
                   num_devices=NCORES)
    xa_in = nc.declare_dram_parameter("xa", [GPC, 4, P], f32, isOutput=False)
    xb_in = nc.declare_dram_parameter("xb", [GPC, 4, P], f32, isOutput=False)
    params = {}
    for l, (C, H) in DIMS.items():
        params[f"wd{l}"] = nc.declare_dram_parameter(f"wd{l}", [C, H], f32, isOutput=False)
        params[f"wb{l}"] = nc.declare_dram_parameter(f"wb{l}", [C, H], f32, isOutput=False)
        params[f"b{l}"] = nc.declare_dram_parameter(f"b{l}", [H, 1], f32, isOutput=False)
    wfc_in = nc.declare_dram_parameter("wfc", [128, 128], f32, isOutput=False)
    bfc_in = nc.declare_dram_parameter("bfc", [128, 1], f32, isOutput=False)
    ptab_in = nc.declare_dram_parameter("ptab", [8, 128, 24], u32, isOutput=False)
    out_d = nc.declare_dram_parameter("out", [128, GPC], f32, isOutput=True)


    state = {}
    for g in range(GPC):
        state[(g, "bm64")] = nc.dram_tensor(f"bm64_{g}", [P, 64], f32)
        state[(g, "bm128")] = nc.dram_tensor(f"bm128_{g}", [P, 128], f32)

    with TileContext(nc) as tc:
        with tc.tile_pool(name="consts", bufs=1) as consts, \
             tc.tile_pool(name="weights", bufs=1) as wpool, \
             tc.tile_pool(name="sbuf", bufs=2) as sbuf, \
             tc.tile_pool(name="psF", bufs=3, space="PSUM") as psF, \
             tc.tile_pool(name="psT", bufs=3, space="PSUM") as psT, \
             tc.tile_pool(name="psB", bufs=2, space="PSUM") as psB:
            pools = {"sbuf": sbuf, "psF": psF, "psT": psT, "psB": psB}
            W = {}
            W["ident"] = consts.tile([128, 128], f32, name="ident")
            masks.make_identity(nc, W["ident"][:])
            W["diagneg"] = consts.tile([128, 128], f32, name="diagneg")
            nc.gpsimd.memset(W["diagneg"][:], 0.0)
            nc.gpsimd.affine_select(
                out=W["diagneg"][:], in_=W["diagneg"][:],
                compare_op=mybir.AluOpType.not_equal, fill=NEG,
                base=0, pattern=[[-1, 128]], channel_multiplier=1)
            W["onescol"] = consts.tile([128, 1], f32, name="onescol")
            nc.vector.memset(W["onescol"][:], 1.0)
            W["pooledT"] = consts.tile([128, GPC], f32, name="pooledT")
            for l, (C, H) in DIMS.items():
                for nm, shp in ((f"wd{l}", [C, H]), (f"wb{l}", [C, H]),
                                (f"b{l}", [H, 1])):
                    tl = wpool.tile(shp, f32, tag=nm, name=nm)
                    nc.sync.dma_start(out=tl[:], in_=params[nm][:, :])
                    W[nm] = tl
            wfc = wpool.tile([128, 128], f32, tag="wfc")
            nc.sync.dma_start(out=wfc[:], in_=wfc_in[:, :])
            bfc = wpool.tile([128, 1], f32, tag="bfc")
            nc.sync.dma_start(out=bfc[:], in_=bfc_in[:, :])

            for g in range(GPC):
                lhs1 = sbuf.tile([4, P], f32, tag="lhs1", bufs=4)
                rhs1 = sbuf.tile([4, P], f32, tag="rhs1", bufs=4)
                nc.sync.dma_start(out=lhs1[:, :], in_=xa_in[g, :, :])
                nc.sync.dma_start(out=rhs1[:, :], in_=xb_in[g, :, :])
                state[(g, "lhs")], state[(g, "rhs")] = lhs1, rhs1

            for l in (1, 2, 3):
                for g in range(GPC):
                    _emit_layer(nc, tc, pools, W, state, g, l, is_last=(l == 3))

            ptf = psT.tile([128, 128], f32, tag="psT")
            nc.tensor.matmul(ptf[:, 0:GPC], wfc[:], W["pooledT"][:, 0:GPC],
                             start=True, stop=True)
            outsb = sbuf.tile([128, GPC], f32, tag="outsb")
            nc.scalar.activation(outsb[:], ptf[:, 0:GPC], RELU, bias=bfc[:])
            nc.sync.dma_start(out=out_d[:, :], in_=outsb[:])

    nc.compile()
    return nc


def _get_nc():
    if "nc" not in _cache:
        _cache["nc"] = _build()
    return _cache["nc"]


def _prep_in_maps(inputs):
    x = np.ascontiguousarray(np.asarray(inputs["x"], dtype=np.float32))
    x = x.reshape(B, P, 3)
    shared = {}
    for l, (C, H) in DIMS.items():
        Wl = np.asarray(inputs[f"W{l}"], dtype=np.float32)
        bl = np.asarray(inputs[f"b{l}"], dtype=np.float32)
        shared[f"wd{l}"] = np.ascontiguousarray(Wl[:C] - Wl[C:])
        shared[f"wb{l}"] = np.ascontiguousarray(Wl[C:])
        shared[f"b{l}"] = np.ascontiguousarray(bl[:, None])
    shared["wfc"] = np.ascontiguousarray(np.asarray(inputs["Wfc"], dtype=np.float32))
    shared["bfc"] = np.ascontiguousarray(
        np.asarray(inputs["bfc"], dtype=np.float32)[:, None])
    xt = x.transpose(0, 2, 1)  # [B, 3, P]
    ones = np.ones((B, 1, P), np.float32)
    sqr = -0.5 * (xt * xt).sum(axis=1, keepdims=True)
    xa = np.concatenate([xt, ones], axis=1)   # [B, 4, P]
    xb = np.concatenate([xt, sqr], axis=1)    # [B, 4, P]
    ptab = np.zeros((8, 128, 24), np.uint32)
    for q in range(128):
        for s in range(KNN):
            j2 = 20 * q + s
            node = (128 * s + q) // 20
            k = (128 * s + q) % 20
            for t in range(8):
                ptab[t, j2 % 128, j2 // 128] = node * 24 + k + 3072 * t
    shared["ptab"] = ptab
    in_maps = []
    for c in range(NCORES):
        m = dict(shared)
        m["xa"] = np.ascontiguousarray(xa[GPC * c:GPC * (c + 1)])
        m["xb"] = np.ascontiguousarray(xb[GPC * c:GPC * (c + 1)])
        in_maps.append(m)
    return in_maps


def _run(inputs, trace=False):
    nc = _get_nc()
    in_maps = _prep_in_maps(inputs)
    res = run_bass_kernel_spmd(nc, in_maps, list(range(NCORES)), trace=trace)
    out = np.concatenate([res.results[c]["out"].T for c in range(NCORES)], axis=0)
    return out.astype(np.float32), res


def kernel(**inputs):
    out, _ = _run(inputs, trace=False)
    return out



# revision 2
# speedup vs baseline: 6.3569x; 6.3569x over previous
"""DGCNN (3x DynamicEdgeConv + global max pool + FC) Trainium2 Bass kernel.

Sharding: data-parallel over graphs. 32 graphs / 8 NeuronCores = 4 graphs/core.
Weights replicated. Each core returns its [128, 4] (feature-major) FC output.

Per-graph algorithm (feature-major [C, P] layout end to end):
  - kNN ranking matrix F = X^T X - 0.5*|x_j|^2 via one PE matmul with the
    lhs=[X;ones], rhs=[X;-0.5 sq] augmentation (top-20 largest F == 20-NN).
  - Top-20 indices per node: 3 rounds of DVE max8 / max_index / match_replace.
  - EdgeConv decomposes: relu(max_k([x_i, x_j-x_i] W + b))
      = relu((Wtop-Wbot)^T x_i + max_k Wbot^T x_j + b)  (relu/max commute).
    So per node: A = Wd^T X (PE), Bm = X^T Wbot rows in DRAM, M = max over the
    20 neighbor rows via 20 indirect DMA gathers with max-accumulate.
  - h^T = relu(transpose(M) + A + b) using PE transpose + matmul accumulated
    into one PSUM tile, ACT applies relu+bias.

HW note: multi-column indirect-DMA offset APs are consumed in a scrambled
order on this hardware, so each gather uses a [128, 1] offset column (one
descriptor per partition — unambiguous, production-tested shape): 20 gathers
per 128-node row-tile into k-slices of a [128, 20, H] tile, then one DVE
tensor_reduce(max) over k.

Runner: the PJRT executable (jit(shard_map(bass_exec))) is built ONCE and
cached; steady-state kernel() calls hit the pjit C++ fast path with only
host->device input transfer + execute + fetch.
"""
import sys

sys.path.insert(0, "/opt/trn_rl_repo")
import numpy as np
import concourse.bass as bass
import concourse.bacc as bacc
import concourse.mybir as mybir
from concourse.bass_utils import run_bass_kernel_spmd
from concourse.tile import TileContext
from concourse import masks

B, P, KNN = 32, 1024, 20
NCORES, GPC = 8, 4
NEG = -3.0e38
f32, u32 = mybir.dt.float32, mybir.dt.uint32
RELU = mybir.ActivationFunctionType.Relu
COPY = mybir.ActivationFunctionType.Copy
MAX = mybir.AluOpType.max
DIMS = {1: (3, 64), 2: (64, 64), 3: (64, 128)}

_cache = {}


def _emit_layer(nc, tc, pools, W, state, g, l, is_last):
    C, H = DIMS[l]
    lhs, rhs = state[(g, "lhs")], state[(g, "rhs")]

    # ---- 1. ones row + sq row (layer 1 rows are shipped from host) ----
    if l > 1:
        _emit_sq_prep(nc, pools, W, lhs, rhs, C)
    _emit_layer_rest(nc, tc, pools, W, state, g, l, is_last)


def _emit_sq_prep(nc, pools, W, lhs, rhs, C):
    psF = pools["psF"]
    pool = pools["sbuf"]
    onescol = W["onescol"]
    nc.vector.memset(lhs[C:C + 1, :], 1.0)
    x2 = pool.tile([C, P], f32, tag="x2", bufs=1)
    nc.scalar.square(x2[0:C, :], lhs[0:C, :])
    for jb in range(2):
        psq = psF.tile([128, 512], f32, tag="psF")
        nc.tensor.matmul(psq[0:1, :], onescol[0:C, :],
                         x2[0:C, 512 * jb:512 * (jb + 1)], start=True, stop=True)
        nc.scalar.activation(rhs[C:C + 1, 512 * jb:512 * (jb + 1)], psq[0:1, :],
                             COPY, scale=-0.5)


def _emit_layer_rest(nc, tc, pools, W, state, g, l, is_last):
    C, H = DIMS[l]
    lhs, rhs = state[(g, "lhs")], state[(g, "rhs")]
    wd, wb, bl = W[f"wd{l}"], W[f"wb{l}"], W[f"b{l}"]
    ident, diagneg, onescol = W["ident"], W["diagneg"], W["onescol"]
    psF, psT, psB = pools["psF"], pools["psT"], pools["psB"]
    pool = pools["sbuf"]
    bm_dram = state[(g, "bm64")] if H == 64 else state[(g, "bm128")]

    # ---- 2. Bm = X^T Wbot, node-major to DRAM ----
    bmt = pool.tile([128, 8, 128], f32, tag="bm", bufs=2)
    for t in range(8):
        pb = psB.tile([128, 128], f32, tag="psB")
        nc.tensor.matmul(pb[:, 0:H], lhs[0:C, 128 * t:128 * (t + 1)], wb[0:C, 0:H],
                         start=True, stop=True)
        nc.scalar.activation(bmt[:, t, 0:H], pb[:, 0:H], COPY)
    nc.sync.dma_start(out=bm_dram[:].rearrange("(t p) h -> p t h", p=128), in_=bmt[:, :, 0:H])

    # ---- 3. F + top-20 indices per node-tile ----
    idxs = pool.tile([128, 8, 24], u32, tag="idx", bufs=3)
    for t in range(8):
        Fsb = pool.tile([128, P], f32, tag="F", bufs=6)
        for jb in range(2):
            fps = psF.tile([128, 512], f32, tag="psF")
            nc.tensor.matmul(fps[:], lhs[0:C + 1, 128 * t:128 * (t + 1)],
                             rhs[0:C + 1, 512 * jb:512 * (jb + 1)],
                             start=True, stop=True)
            nc.scalar.activation(Fsb[:, 512 * jb:512 * (jb + 1)], fps[:], COPY)
        nc.vector.tensor_add(Fsb[:, 128 * t:128 * (t + 1)],
                             Fsb[:, 128 * t:128 * (t + 1)], diagneg[:])
        for r in range(3):
            m8 = pool.tile([128, 8], f32, tag="m8", bufs=4)
            nc.vector.max(out=m8, in_=Fsb)
            nc.vector.max_index(out=idxs[:, t, 8 * r:8 * r + 8], in_max=m8,
                                in_values=Fsb)
            if r < 2:
                nc.vector.match_replace(out=Fsb, in_to_replace=m8, in_values=Fsb,
                                        imm_value=NEG)

    # ---- 4+5. per-row-tile: 20 single-descriptor-per-partition gathers ----
    if is_last:
        h3 = pool.tile([128, P], f32, tag="h3", bufs=1)
        dst = h3
    else:
        Cn = H + 1
        lhs_n = pool.tile([Cn, P], f32, tag=f"lhs{l + 1}", bufs=4)
        rhs_n = pool.tile([Cn, P], f32, tag=f"rhs{l + 1}", bufs=4)
        dst = lhs_n
    for t in range(8):
        wt = pool.tile([128, 24], u32, tag="wt", bufs=6, name=f"wt{t}")
        nc.vector.tensor_copy(wt[:], idxs[:, t, :])
        gt = pool.tile([128, KNN, H], f32, tag="gt", bufs=4, name=f"gt{t}")
        for k in range(KNN):
            nc.gpsimd.indirect_dma_start(
                out=gt[:, k, :], out_offset=None,
                in_=bm_dram[:, :],
                in_offset=bass.IndirectOffsetOnAxis(ap=wt[:, k:k + 1], axis=0),
                bounds_check=P - 1, oob_is_err=False)
        Mt = pool.tile([128, H], f32, tag="Mt", bufs=6, name=f"Mt{t}")
        nc.vector.tensor_reduce(
            out=Mt[:], in_=gt[:].rearrange("p c h -> p h c"),
            axis=mybir.AxisListType.X, op=MAX)
        pt = psT.tile([128, 128], f32, tag="psT")
        nc.tensor.matmul(pt[0:H, :], Mt[:], ident[:], is_transpose=True,
                         start=True, stop=False)
        nc.tensor.matmul(pt[0:H, :], wd[0:C, 0:H],
                         lhs[0:C, 128 * t:128 * (t + 1)], start=False, stop=True)
        nc.scalar.activation(dst[0:H, 128 * t:128 * (t + 1)], pt[0:H, :], RELU,
                             bias=bl[0:H, :])

    if is_last:
        nc.vector.tensor_reduce(out=W["pooledT"][:, g:g + 1], in_=h3[:],
                                axis=mybir.AxisListType.X, op=MAX)
    else:
        nc.scalar.activation(rhs_n[0:H, 0:512], lhs_n[0:H, 0:512], COPY)
        nc.scalar.activation(rhs_n[0:H, 512:P], lhs_n[0:H, 512:P], COPY)
        state[(g, "lhs")], state[(g, "rhs")] = lhs_n, rhs_n


def _build():
    nc = bacc.Bacc("TRN2", target_bir_lowering=False, debug=False,
                   num_devices=NCORES)
    xa_in = nc.declare_dram_parameter("xa", [GPC, 4, P], f32, isOutput=False)
    xb_in = nc.declare_dram_parameter("xb", [GPC, 4, P], f32, isOutput=False)
    params = {}
    for l, (C, H) in DIMS.items():
        params[f"wd{l}"] = nc.declare_dram_parameter(f"wd{l}", [C, H], f32, isOutput=False)
        params[f"wb{l}"] = nc.declare_dram_parameter(f"wb{l}", [C, H], f32, isOutput=False)
        params[f"b{l}"] = nc.declare_dram_parameter(f"b{l}", [H, 1], f32, isOutput=False)
    wfc_in = nc.declare_dram_parameter("wfc", [128, 128], f32, isOutput=False)
    bfc_in = nc.declare_dram_parameter("bfc", [128, 1], f32, isOutput=False)
    ptab_in = nc.declare_dram_parameter("ptab", [8, 128, 24], u32, isOutput=False)
    out_d = nc.declare_dram_parameter("out", [128, GPC], f32, isOutput=True)


    state = {}
    for g in range(GPC):
        state[(g, "bm64")] = nc.dram_tensor(f"bm64_{g}", [P, 64], f32)
        state[(g, "bm128")] = nc.dram_tensor(f"bm128_{g}", [P, 128], f32)

    with TileContext(nc) as tc:
        with tc.tile_pool(name="consts", bufs=1) as consts, \
             tc.tile_pool(name="weights", bufs=1) as wpool, \
             tc.tile_pool(name="sbuf", bufs=2) as sbuf, \
             tc.tile_pool(name="psF", bufs=3, space="PSUM") as psF, \
             tc.tile_pool(name="psT", bufs=3, space="PSUM") as psT, \
             tc.tile_pool(name="psB", bufs=2, space="PSUM") as psB:
            pools = {"sbuf": sbuf, "psF": psF, "psT": psT, "psB": psB}
            W = {}
            W["ident"] = consts.tile([128, 128], f32, name="ident")
            masks.make_identity(nc, W["ident"][:])
            W["diagneg"] = consts.tile([128, 128], f32, name="diagneg")
            nc.gpsimd.memset(W["diagneg"][:], 0.0)
            nc.gpsimd.affine_select(
                out=W["diagneg"][:], in_=W["diagneg"][:],
                compare_op=mybir.AluOpType.not_equal, fill=NEG,
                base=0, pattern=[[-1, 128]], channel_multiplier=1)
            W["onescol"] = consts.tile([128, 1], f32, name="onescol")
            nc.vector.memset(W["onescol"][:], 1.0)
            W["pooledT"] = consts.tile([128, GPC], f32, name="pooledT")
            for l, (C, H) in DIMS.items():
                for nm, shp in ((f"wd{l}", [C, H]), (f"wb{l}", [C, H]),
                                (f"b{l}", [H, 1])):
                    tl = wpool.tile(shp, f32, tag=nm, name=nm)
                    nc.sync.dma_start(out=tl[:], in_=params[nm][:, :])
                    W[nm] = tl
            wfc = wpool.tile([128, 128], f32, tag="wfc")
            nc.sync.dma_start(out=wfc[:], in_=wfc_in[:, :])
            bfc = wpool.tile([128, 1], f32, tag="bfc")
            nc.sync.dma_start(out=bfc[:], in_=bfc_in[:, :])

            for g in range(GPC):
                lhs1 = sbuf.tile([4, P], f32, tag="lhs1", bufs=4)
                rhs1 = sbuf.tile([4, P], f32, tag="rhs1", bufs=4)
                nc.sync.dma_start(out=lhs1[:, :], in_=xa_in[g, :, :])
                nc.sync.dma_start(out=rhs1[:, :], in_=xb_in[g, :, :])
                state[(g, "lhs")], state[(g, "rhs")] = lhs1, rhs1

            for l in (1, 2, 3):
                for g in range(GPC):
                    _emit_layer(nc, tc, pools, W, state, g, l, is_last=(l == 3))

            ptf = psT.tile([128, 128], f32, tag="psT")
            nc.tensor.matmul(ptf[:, 0:GPC], wfc[:], W["pooledT"][:, 0:GPC],
                             start=True, stop=True)
            outsb = sbuf.tile([128, GPC], f32, tag="outsb")
            nc.scalar.activation(outsb[:], ptf[:, 0:GPC], RELU, bias=bfc[:])
            nc.sync.dma_start(out=out_d[:, :], in_=outsb[:])

    nc.compile()
    return nc


def _get_nc():
    if "nc" not in _cache:
        _cache["nc"] = _build()
    return _cache["nc"]


def _ptab():
    if "ptab" not in _cache:
        ptab = np.zeros((8, 128, 24), np.uint32)
        for q in range(128):
            for s in range(KNN):
                j2 = 20 * q + s
                node = (128 * s + q) // 20
                k = (128 * s + q) % 20
                for t in range(8):
                    ptab[t, j2 % 128, j2 // 128] = node * 24 + k + 3072 * t
        _cache["ptab"] = ptab
    return _cache["ptab"]


def _prep_shared(inputs):
    shared = {}
    for l, (C, H) in DIMS.items():
        Wl = np.asarray(inputs[f"W{l}"], dtype=np.float32)
        bl = np.asarray(inputs[f"b{l}"], dtype=np.float32)
        shared[f"wd{l}"] = np.ascontiguousarray(Wl[:C] - Wl[C:])
        shared[f"wb{l}"] = np.ascontiguousarray(Wl[C:])
        shared[f"b{l}"] = np.ascontiguousarray(bl[:, None])
    shared["wfc"] = np.ascontiguousarray(np.asarray(inputs["Wfc"], dtype=np.float32))
    shared["bfc"] = np.ascontiguousarray(
        np.asarray(inputs["bfc"], dtype=np.float32)[:, None])
    shared["ptab"] = _ptab()
    return shared


def _prep_xaxb(inputs):
    x = np.ascontiguousarray(np.asarray(inputs["x"], dtype=np.float32))
    x = x.reshape(B, P, 3)
    xt = x.transpose(0, 2, 1)  # [B, 3, P]
    ones = np.ones((B, 1, P), np.float32)
    sqr = -0.5 * (xt * xt).sum(axis=1, keepdims=True)
    xa = np.concatenate([xt, ones], axis=1)   # [B, 4, P]
    xb = np.concatenate([xt, sqr], axis=1)    # [B, 4, P]
    return xa, xb


def _prep_in_maps(inputs):
    shared = _prep_shared(inputs)
    xa, xb = _prep_xaxb(inputs)
    in_maps = []
    for c in range(NCORES):
        m = dict(shared)
        m["xa"] = np.ascontiguousarray(xa[GPC * c:GPC * (c + 1)])
        m["xb"] = np.ascontiguousarray(xb[GPC * c:GPC * (c + 1)])
        in_maps.append(m)
    return in_maps


def _get_runner():
    """Build the jit(shard_map(bass_exec)) executable once; cache it.

    Mirrors concourse.bass2jax.run_bass_via_pjrt's multi-core path, but
    hoists everything static out of the per-call path so repeated calls hit
    the pjit C++ fast path (no re-trace / re-lower / re-compile).
    """
    if "runner" in _cache:
        return _cache["runner"]
    import jax
    from jax.experimental.shard_map import shard_map
    from jax.sharding import Mesh, PartitionSpec
    from concourse import bass2jax

    nc = _get_nc()
    bass2jax.install_neuronx_cc_hook()
    assert nc.dbg_addr is None and not nc.dbg_callbacks

    partition_name = nc.partition_id_tensor.name if nc.partition_id_tensor else None
    in_names, out_names, out_avals, zero_shapes = [], [], [], []
    for alloc in nc.m.functions[0].allocations:
        if not isinstance(alloc, mybir.MemoryLocationSet):
            continue
        name = alloc.memorylocations[0].name
        if alloc.kind == "ExternalInput":
            if name != partition_name:
                in_names.append(name)
        elif alloc.kind == "ExternalOutput":
            shape = tuple(alloc.tensor_shape)
            dtype = mybir.dt.np(alloc.dtype)
            out_names.append(name)
            out_avals.append(jax.core.ShapedArray(shape, dtype))
            zero_shapes.append((shape, dtype))
    n_params = len(in_names)
    n_outs = len(out_names)
    all_in_names = list(in_names) + list(out_names)
    if partition_name is not None:
        all_in_names.append(partition_name)

    def _body(*args):
        operands = list(args)
        if partition_name is not None:
            operands.append(bass2jax.partition_id_tensor())
        outs = bass2jax._bass_exec_p.bind(
            *operands,
            out_avals=tuple(out_avals),
            in_names=tuple(all_in_names),
            out_names=tuple(out_names),
            lowering_input_output_aliases=(),
            sim_require_finite=True,
            sim_require_nnan=True,
            nc=nc,
        )
        return tuple(outs)

    devices = jax.devices()[:NCORES]
    assert len(devices) == NCORES
    mesh = Mesh(np.asarray(devices), ("core",))
    in_specs = (PartitionSpec("core"),) * (n_params + n_outs)
    out_specs = (PartitionSpec("core"),) * n_outs
    donate = tuple(range(n_params, n_params + n_outs))
    sharded = jax.jit(
        shard_map(_body, mesh=mesh, in_specs=in_specs, out_specs=out_specs,
                  check_rep=False),
        donate_argnums=donate, keep_unused=True)
    _cache["runner"] = (sharded, in_names, out_names, zero_shapes)
    return _cache["runner"]


def _prep_global(inputs):
    """Global (concat-over-cores along axis 0) input arrays, keyed by name."""
    shared = _prep_shared(inputs)
    xa, xb = _prep_xaxb(inputs)
    g = {"xa": xa.reshape(NCORES * GPC, 4, P),
         "xb": xb.reshape(NCORES * GPC, 4, P)}
    for k, v in shared.items():
        g[k] = np.tile(v, (NCORES,) + (1,) * (v.ndim - 1))
    return g


def kernel(**inputs):
    sharded, in_names, out_names, zero_shapes = _get_runner()
    gin = _prep_global(inputs)
    args = [gin[name] for name in in_names]
    zeros = [np.zeros((NCORES * s[0],) + tuple(s[1:]), d) for s, d in zero_shapes]
    out_arrs = sharded(*args, *zeros)
    out = np.asarray(out_arrs[out_names.index("out")])  # [8*128, GPC]
    return np.ascontiguousarray(
        out.reshape(NCORES, 128, GPC).transpose(0, 2, 1).reshape(B, 128))


class _Res:
    exec_time_ns = None
    results = None


def _run(inputs, trace=False):
    if trace:
        nc = _get_nc()
        in_maps = _prep_in_maps(inputs)
        res = run_bass_kernel_spmd(nc, in_maps, list(range(NCORES)), trace=True)
        out = np.concatenate([res.results[c]["out"].T for c in range(NCORES)],
                             axis=0)
        return out.astype(np.float32), res
    return kernel(**inputs), _Res()


# revision 4
# speedup vs baseline: 10.1059x; 1.5898x over previous
"""DGCNN (3x DynamicEdgeConv + global max pool + FC) Trainium2 Bass kernel.

Sharding: data-parallel over graphs. 32 graphs / 8 NeuronCores = 4 graphs/core.
Weights replicated. Each core returns its [128, 4] (feature-major) FC output.

Per-graph algorithm (feature-major [C, P] layout end to end):
  - kNN ranking matrix F = X^T X - 0.5*|x_j|^2 via two accumulating PE
    matmuls into one PSUM tile: X^T X, then ones^T * (-0.5|x|^2 row)
    (top-20 largest F == 20-NN).
  - Top-20 indices per node: 3 rounds of DVE max8 / max_index / match_replace.
  - EdgeConv decomposes: relu(max_k([x_i, x_j-x_i] W + b))
      = relu((Wtop-Wbot)^T x_i + max_k Wbot^T x_j + b)  (relu/max commute).
    So per node: A = Wd^T X (PE), Bm = X^T Wbot rows in DRAM, M = max over the
    20 neighbor rows via 20 indirect DMA gathers with max-accumulate.
  - h^T = relu(transpose(M) + A + b) using PE transpose + matmul accumulated
    into one PSUM tile, ACT applies relu+bias.

HW note: multi-column indirect-DMA offset APs are consumed in a scrambled
order on this hardware, so each gather uses a [128, 1] offset column (one
descriptor per partition — unambiguous, production-tested shape): 20 gathers
per 128-node row-tile into k-slices of a [128, 20, H] tile, then one DVE
tensor_reduce(max) over k.

Runner: the PJRT executable (jit(shard_map(bass_exec))) is built ONCE and
cached; steady-state kernel() calls hit the pjit C++ fast path with only
host->device input transfer + execute + fetch. Only x is sharded over the
core mesh axis; all weights are replicated (transferred once, not 8x).
"""
import sys

sys.path.insert(0, "/opt/trn_rl_repo")
import numpy as np
import concourse.bass as bass
import concourse.bacc as bacc
import concourse.mybir as mybir
from concourse.bass_utils import run_bass_kernel_spmd
from concourse.tile import TileContext
from concourse import masks

B, P, KNN = 32, 1024, 20
NCORES, GPC = 8, 4
NEG = -3.0e38
f32, u32 = mybir.dt.float32, mybir.dt.uint32
RELU = mybir.ActivationFunctionType.Relu
COPY = mybir.ActivationFunctionType.Copy
MAX = mybir.AluOpType.max
DIMS = {1: (3, 64), 2: (64, 64), 3: (64, 128)}
SHARDED_INPUTS = {"x"}

_cache = {}


def _emit_sq_prep(nc, pools, W, lhs, sqrow, C):
    """sqrow[0, q] = -0.5 * |x_q|^2 from the feature-major lhs [C, P]."""
    psF = pools["psF"]
    pool = pools["sbuf"]
    onescol = W["onescol"]
    x2 = pool.tile([C, P], f32, tag="x2", bufs=1)
    nc.scalar.square(x2[0:C, :], lhs[0:C, :])
    for jb in range(2):
        psq = psF.tile([128, 512], f32, tag="psF")
        nc.tensor.matmul(psq[0:1, :], onescol[0:C, :],
                         x2[0:C, 512 * jb:512 * (jb + 1)], start=True, stop=True)
        nc.scalar.activation(sqrow[0:1, 512 * jb:512 * (jb + 1)], psq[0:1, :],
                             COPY, scale=-0.5)


def _emit_layer(nc, tc, pools, W, state, g, l, is_last):
    C, H = DIMS[l]
    lhs = state[(g, "lhs")]
    wd, wb, bl = W[f"wd{l}"], W[f"wb{l}"], W[f"b{l}"]
    ident, diagneg, ones128 = W["ident"], W["diagneg"], W["ones128"]
    psF, psT, psB = pools["psF"], pools["psT"], pools["psB"]
    pool = pools["sbuf"]
    bm_dram = state[(g, "bm64")] if H == 64 else state[(g, "bm128")]

    # ---- 1. sq row ----
    sqrow = pool.tile([1, P], f32, tag="sq", bufs=2)
    _emit_sq_prep(nc, pools, W, lhs, sqrow, C)

    # ---- 2. Bm = X^T Wbot, node-major to DRAM ----
    bmt = pool.tile([128, 8, 128], f32, tag="bm", bufs=2)
    for t in range(8):
        pb = psB.tile([128, 128], f32, tag="psB")
        nc.tensor.matmul(pb[:, 0:H], lhs[0:C, 128 * t:128 * (t + 1)], wb[0:C, 0:H],
                         start=True, stop=True)
        nc.scalar.activation(bmt[:, t, 0:H], pb[:, 0:H], COPY)
    nc.sync.dma_start(out=bm_dram[:].rearrange("(t p) h -> p t h", p=128), in_=bmt[:, :, 0:H])

    # ---- 3. F + top-20 indices per node-tile ----
    idxs = pool.tile([128, 8, 24], u32, tag="idx", bufs=3)
    for t in range(8):
        Fsb = pool.tile([128, P], f32, tag="F", bufs=6)
        for jb in range(2):
            fps = psF.tile([128, 512], f32, tag="psF")
            nc.tensor.matmul(fps[:], lhs[0:C, 128 * t:128 * (t + 1)],
                             lhs[0:C, 512 * jb:512 * (jb + 1)],
                             start=True, stop=False)
            nc.tensor.matmul(fps[:], ones128[0:1, :],
                             sqrow[0:1, 512 * jb:512 * (jb + 1)],
                             start=False, stop=True)
            nc.scalar.activation(Fsb[:, 512 * jb:512 * (jb + 1)], fps[:], COPY)
        nc.vector.tensor_add(Fsb[:, 128 * t:128 * (t + 1)],
                             Fsb[:, 128 * t:128 * (t + 1)], diagneg[:])
        for r in range(3):
            m8 = pool.tile([128, 8], f32, tag="m8", bufs=4)
            nc.vector.max(out=m8, in_=Fsb)
            nc.vector.max_index(out=idxs[:, t, 8 * r:8 * r + 8], in_max=m8,
                                in_values=Fsb)
            if r < 2:
                nc.vector.match_replace(out=Fsb, in_to_replace=m8, in_values=Fsb,
                                        imm_value=NEG)

    # ---- 4+5. per-row-tile: 20 single-descriptor-per-partition gathers ----
    if is_last:
        h3 = pool.tile([128, P], f32, tag="h3", bufs=1)
        dst = h3
    else:
        lhs_n = pool.tile([H, P], f32, tag=f"lhs{l + 1}", bufs=4)
        dst = lhs_n
    for t in range(8):
        wt = pool.tile([128, 24], u32, tag="wt", bufs=6, name=f"wt{t}")
        nc.vector.tensor_copy(wt[:], idxs[:, t, :])
        gt = pool.tile([128, KNN, H], f32, tag="gt", bufs=4, name=f"gt{t}")
        for k in range(KNN):
            nc.gpsimd.indirect_dma_start(
                out=gt[:, k, :], out_offset=None,
                in_=bm_dram[:, :],
                in_offset=bass.IndirectOffsetOnAxis(ap=wt[:, k:k + 1], axis=0),
                bounds_check=P - 1, oob_is_err=False)
        Mt = pool.tile([128, H], f32, tag="Mt", bufs=6, name=f"Mt{t}")
        nc.vector.tensor_reduce(
            out=Mt[:], in_=gt[:].rearrange("p c h -> p h c"),
            axis=mybir.AxisListType.X, op=MAX)
        pt = psT.tile([128, 128], f32, tag="psT")
        nc.tensor.matmul(pt[0:H, :], Mt[:], ident[:], is_transpose=True,
                         start=True, stop=False)
        nc.tensor.matmul(pt[0:H, :], wd[0:C, 0:H],
                         lhs[0:C, 128 * t:128 * (t + 1)], start=False, stop=True)
        nc.scalar.activation(dst[0:H, 128 * t:128 * (t + 1)], pt[0:H, :], RELU,
                             bias=bl[0:H, :])

    if is_last:
        nc.vector.tensor_reduce(out=W["pooledT"][:, g:g + 1], in_=h3[:],
                                axis=mybir.AxisListType.X, op=MAX)
    else:
        state[(g, "lhs")] = lhs_n


def _build():
    nc = bacc.Bacc("TRN2", target_bir_lowering=False, debug=False,
                   num_devices=NCORES)
    x_in = nc.declare_dram_parameter("x", [GPC, 3, P], f32, isOutput=False)
    params = {}
    for l, (C, H) in DIMS.items():
        params[f"wd{l}"] = nc.declare_dram_parameter(f"wd{l}", [C, H], f32, isOutput=False)
        params[f"wb{l}"] = nc.declare_dram_parameter(f"wb{l}", [C, H], f32, isOutput=False)
        params[f"b{l}"] = nc.declare_dram_parameter(f"b{l}", [H, 1], f32, isOutput=False)
    wfc_in = nc.declare_dram_parameter("wfc", [128, 128], f32, isOutput=False)
    bfc_in = nc.declare_dram_parameter("bfc", [128, 1], f32, isOutput=False)
    out_d = nc.declare_dram_parameter("out", [128, GPC], f32, isOutput=True)

    state = {}
    for g in range(GPC):
        state[(g, "bm64")] = nc.dram_tensor(f"bm64_{g}", [P, 64], f32)
        state[(g, "bm128")] = nc.dram_tensor(f"bm128_{g}", [P, 128], f32)

    with TileContext(nc) as tc:
        with tc.tile_pool(name="consts", bufs=1) as consts, \
             tc.tile_pool(name="weights", bufs=1) as wpool, \
             tc.tile_pool(name="sbuf", bufs=2) as sbuf, \
             tc.tile_pool(name="psF", bufs=3, space="PSUM") as psF, \
             tc.tile_pool(name="psT", bufs=3, space="PSUM") as psT, \
             tc.tile_pool(name="psB", bufs=2, space="PSUM") as psB:
            pools = {"sbuf": sbuf, "psF": psF, "psT": psT, "psB": psB}
            W = {}
            W["ident"] = consts.tile([128, 128], f32, name="ident")
            masks.make_identity(nc, W["ident"][:])
            W["diagneg"] = consts.tile([128, 128], f32, name="diagneg")
            nc.gpsimd.memset(W["diagneg"][:], 0.0)
            nc.gpsimd.affine_select(
                out=W["diagneg"][:], in_=W["diagneg"][:],
                compare_op=mybir.AluOpType.not_equal, fill=NEG,
                base=0, pattern=[[-1, 128]], channel_multiplier=1)
            W["onescol"] = consts.tile([128, 1], f32, name="onescol")
            nc.vector.memset(W["onescol"][:], 1.0)
            W["ones128"] = consts.tile([1, 128], f32, name="ones128")
            nc.vector.memset(W["ones128"][:], 1.0)
            W["pooledT"] = consts.tile([128, GPC], f32, name="pooledT")
            for l, (C, H) in DIMS.items():
                for nm, shp in ((f"wd{l}", [C, H]), (f"wb{l}", [C, H]),
                                (f"b{l}", [H, 1])):
                    tl = wpool.tile(shp, f32, tag=nm, name=nm)
                    nc.sync.dma_start(out=tl[:], in_=params[nm][:, :])
                    W[nm] = tl
            wfc = wpool.tile([128, 128], f32, tag="wfc")
            nc.sync.dma_start(out=wfc[:], in_=wfc_in[:, :])
            bfc = wpool.tile([128, 1], f32, tag="bfc")
            nc.sync.dma_start(out=bfc[:], in_=bfc_in[:, :])

            for g in range(GPC):
                lhs1 = sbuf.tile([3, P], f32, tag="lhs1", bufs=4)
                nc.sync.dma_start(out=lhs1[:, :], in_=x_in[g, :, :])
                state[(g, "lhs")] = lhs1

            for l in (1, 2, 3):
                for g in range(GPC):
                    _emit_layer(nc, tc, pools, W, state, g, l, is_last=(l == 3))

            ptf = psT.tile([128, 128], f32, tag="psT")
            nc.tensor.matmul(ptf[:, 0:GPC], wfc[:], W["pooledT"][:, 0:GPC],
                             start=True, stop=True)
            outsb = sbuf.tile([128, GPC], f32, tag="outsb")
            nc.scalar.activation(outsb[:], ptf[:, 0:GPC], RELU, bias=bfc[:])
            nc.sync.dma_start(out=out_d[:, :], in_=outsb[:])

    nc.compile()
    return nc


def _get_nc():
    if "nc" not in _cache:
        _cache["nc"] = _build()
    return _cache["nc"]


def _prep_shared(inputs):
    shared = {}
    for l, (C, H) in DIMS.items():
        Wl = np.asarray(inputs[f"W{l}"], dtype=np.float32)
        bl = np.asarray(inputs[f"b{l}"], dtype=np.float32)
        shared[f"wd{l}"] = np.ascontiguousarray(Wl[:C] - Wl[C:])
        shared[f"wb{l}"] = np.ascontiguousarray(Wl[C:])
        shared[f"b{l}"] = np.ascontiguousarray(bl[:, None])
    shared["wfc"] = np.ascontiguousarray(np.asarray(inputs["Wfc"], dtype=np.float32))
    shared["bfc"] = np.ascontiguousarray(
        np.asarray(inputs["bfc"], dtype=np.float32)[:, None])
    return shared


def _prep_x(inputs):
    x = np.asarray(inputs["x"], dtype=np.float32).reshape(B, P, 3)
    return np.ascontiguousarray(x.transpose(0, 2, 1))  # [B, 3, P]


def _prep_in_maps(inputs):
    shared = _prep_shared(inputs)
    xt = _prep_x(inputs)
    in_maps = []
    for c in range(NCORES):
        m = dict(shared)
        m["x"] = np.ascontiguousarray(xt[GPC * c:GPC * (c + 1)])
        in_maps.append(m)
    return in_maps


def _get_runner():
    """Build the jit(shard_map(bass_exec)) executable once; cache it.

    Mirrors concourse.bass2jax.run_bass_via_pjrt's multi-core path, but
    hoists everything static out of the per-call path so repeated calls hit
    the pjit C++ fast path (no re-trace / re-lower / re-compile). Only x and
    the donated output-zero buffers are sharded over the mesh axis; weights
    use a replicated spec so the host->device transfer ships one copy.
    """
    if "runner" in _cache:
        return _cache["runner"]
    import jax
    from jax.experimental.shard_map import shard_map
    from jax.sharding import Mesh, PartitionSpec
    from concourse import bass2jax

    nc = _get_nc()
    bass2jax.install_neuronx_cc_hook()
    assert nc.dbg_addr is None and not nc.dbg_callbacks

    partition_name = nc.partition_id_tensor.name if nc.partition_id_tensor else None
    in_names, out_names, out_avals, zero_shapes = [], [], [], []
    for alloc in nc.m.functions[0].allocations:
        if not isinstance(alloc, mybir.MemoryLocationSet):
            continue
        name = alloc.memorylocations[0].name
        if alloc.kind == "ExternalInput":
            if name != partition_name:
                in_names.append(name)
        elif alloc.kind == "ExternalOutput":
            shape = tuple(alloc.tensor_shape)
            dtype = mybir.dt.np(alloc.dtype)
            out_names.append(name)
            out_avals.append(jax.core.ShapedArray(shape, dtype))
            zero_shapes.append((shape, dtype))
    n_params = len(in_names)
    n_outs = len(out_names)
    all_in_names = list(in_names) + list(out_names)
    if partition_name is not None:
        all_in_names.append(partition_name)

    def _body(*args):
        operands = list(args)
        if partition_name is not None:
            operands.append(bass2jax.partition_id_tensor())
        outs = bass2jax._bass_exec_p.bind(
            *operands,
            out_avals=tuple(out_avals),
            in_names=tuple(all_in_names),
            out_names=tuple(out_names),
            lowering_input_output_aliases=(),
            sim_require_finite=True,
            sim_require_nnan=True,
            nc=nc,
        )
        return tuple(outs)

    devices = jax.devices()[:NCORES]
    assert len(devices) == NCORES
    mesh = Mesh(np.asarray(devices), ("core",))
    in_specs = tuple(
        PartitionSpec("core") if nm in SHARDED_INPUTS else PartitionSpec()
        for nm in in_names) + (PartitionSpec("core"),) * n_outs
    out_specs = (PartitionSpec("core"),) * n_outs
    donate = tuple(range(n_params, n_params + n_outs))
    sharded = jax.jit(
        shard_map(_body, mesh=mesh, in_specs=in_specs, out_specs=out_specs,
                  check_rep=False),
        donate_argnums=donate, keep_unused=True)
    _cache["runner"] = (sharded, in_names, out_names, zero_shapes)
    return _cache["runner"]


def _prep_global(inputs):
    """Input arrays keyed by name: x concat-over-cores on axis 0 (sharded);
    weights at their per-core shape (replicated spec ships one copy)."""
    g = _prep_shared(inputs)
    g["x"] = _prep_x(inputs)  # [NCORES*GPC, 3, P]
    return g


def kernel(**inputs):
    sharded, in_names, out_names, zero_shapes = _get_runner()
    gin = _prep_global(inputs)
    args = [gin[name] for name in in_names]
    zeros = [np.zeros((NCORES * s[0],) + tuple(s[1:]), d) for s, d in zero_shapes]
    out_arrs = sharded(*args, *zeros)
    out = np.asarray(out_arrs[out_names.index("out")])  # [8*128, GPC]
    return np.ascontiguousarray(
        out.reshape(NCORES, 128, GPC).transpose(0, 2, 1).reshape(B, 128))


class _Res:
    exec_time_ns = None
    results = None


def _run(inputs, trace=False):
    if trace:
        nc = _get_nc()
        in_maps = _prep_in_maps(inputs)
        res = run_bass_kernel_spmd(nc, in_maps, list(range(NCORES)), trace=True)
        out = np.concatenate([res.results[c]["out"].T for c in range(NCORES)],
                             axis=0)
        return out.astype(np.float32), res
    return kernel(**inputs), _Res()
